# revision 65
# baseline (speedup 1.0000x reference)
"""AttentionSubsample Trainium2 kernel — data-parallel over batch on 8 cores.

v18 (final, ~174us; v4 baseline was 335us).  Design:
  - v-bias folded into the V tiles at evacuation (attn@(v + tvb) =
    o + z*tvb, so the /z normalization yields y = o/z + tvb exactly);
    this removes the gpsimd add from the y-chain latency path.
  - Transpose-free scores: scores are computed TRANSPOSED ([keys, (h,q)])
    so attn@v needs no attention transpose.  scoresT per (item,
    key-chunk c) is ONE matmul: lhsT = qkT key-chunk [128 (h,d) rows,
    128 key cols] (bf16, FWL, full K=128 contraction), rhs = a
    block-diagonal query tile qbd [128 (h,d), (h,q)=392] built by a DVE
    broadcast-multiply of the q columns with a 0/1 block-mask constant
    (this also kills the v4 qka partition-shuffle DMAs).
  - rel-pos bias is preloaded into PSUM by an identity matmul
    (lhsT=I128, rhs=abT const), scoresT accumulates on top.  Pad keys
    (196..255, incl. the q-column overlap of the c=1 window) get bias
    -1e6 -> exp -> 0, which gates every garbage contribution.
  - attn@v: lhsT = attnT slices directly (64 MMs, N=33, ones-column
    gives the softmax denominators for free); 1/WS fold in the y-chain.
  - fin: 4 PE transposes (hardswish output) + 8 proj MMs (N=512).
  - skew-2 pipeline A+scores(it) | attn@v(it-1) | fin(it-2) with stage
    chunks interleaved for uniform PE duty: the HAM clock gate throttles
    to 1.2GHz whenever MID-window duty sags, not just on full idle.
  - 5 PSUM pools sized to exactly 8 banks so attn@v never WAR-waits on
    fin's evacuation: kq(1) v(1) scores(2) av(2, shares with hsT
    transposes) proj(2).
"""

import numpy as np
import ml_dtypes

import concourse.bass as bass
import concourse.tile as tile
from concourse import bacc, mybir
from concourse.bass_utils import run_bass_kernel_spmd

BF16 = mybir.dt.bfloat16
F32 = mybir.dt.float32
F8 = mybir.dt.float8e4
WS = 32.0          # fp8 weight scale; folded out via exp scale & v evac

B, N, NQ, C = 512, 196, 49, 256
H = 8
NCORES = 8
BPC = B // NCORES
EPS = 1e-5
SCALE = 16 ** -0.5
AF = mybir.ActivationFunctionType
ALU = mybir.AluOpType

bf16 = ml_dtypes.bfloat16
STAGE_MARKS = []


def build_core(nbatch=BPC):
    assert nbatch % 4 == 0
    nc = bacc.Bacc("TRN2", target_bir_lowering=False, debug=False)

    xt_d = nc.dram_tensor("xt", [nbatch, 2, 128, N], F8, kind="ExternalInput")
    wkt_d = nc.dram_tensor("wkt", [2, 128, 128], F8, kind="ExternalInput")
    wqt_d = nc.dram_tensor("wqt", [2, 128, 128], F8, kind="ExternalInput")
    wvt_d = nc.dram_tensor("wvt", [2, 128, 256], F8, kind="ExternalInput")
    wpt_d = nc.dram_tensor("wpt", [2, 128, 512], BF16, kind="ExternalInput")
    tkq_d = nc.dram_tensor("tkq", [128, 245], BF16, kind="ExternalInput")
    abt_d = nc.dram_tensor("abt", [2, 128, 392], BF16, kind="ExternalInput")
    mask_d = nc.dram_tensor("mask", [128, 392], BF16, kind="ExternalInput")
    id128_d = nc.dram_tensor("id128", [128, 128], BF16, kind="ExternalInput")
    tvb_d = nc.dram_tensor("tvb", [128, 256], F32, kind="ExternalInput")
    out_d = nc.dram_tensor("out", [nbatch, 49, 512], BF16, kind="ExternalOutput")
    outv = out_d.rearrange("(x pr b2) q o -> b2 x q pr o", pr=2, b2=2)

    with tile.TileContext(nc) as tc:
        with (
            tc.tile_pool(name="consts", bufs=1) as consts,
            tc.tile_pool(name="io", bufs=4) as io,
            tc.tile_pool(name="work", bufs=8) as work,
            tc.tile_pool(name="stackp", bufs=1) as stackp,
            tc.tile_pool(name="ps_kq", bufs=1, space="PSUM") as ps_kq,
            tc.tile_pool(name="ps_v", bufs=1, space="PSUM") as ps_v,
            tc.tile_pool(name="ps_s", bufs=2, space="PSUM") as ps_s,
            tc.tile_pool(name="ps_av", bufs=2, space="PSUM") as ps_av,
            tc.tile_pool(name="ps_pj", bufs=1, space="PSUM") as ps_pj,
        ):
            # A-phase consts first so quad 0 is not gated on B-phase blobs
            wkt_sb = consts.tile([128, 2, 128], F8)
            wqt_sb = consts.tile([128, 2, 128], F8)
            wvt_sb = consts.tile([128, 2, 256], F8)
            wpt_sb = consts.tile([128, 2, 512], BF16)
            tkq_sb = consts.tile([128, 245], BF16)
            abt_sb = consts.tile([128, 2, 392], BF16)
            mask_sb = consts.tile([128, 392], BF16)
            id128_sb = consts.tile([128, 128], BF16)
            tvb_sb = consts.tile([128, 256], F32)
            for c in range(2):
                nc.scalar.dma_start(out=wkt_sb[:, c, :], in_=wkt_d[c])
                nc.scalar.dma_start(out=wqt_sb[:, c, :], in_=wqt_d[c])
            nc.scalar.dma_start(out=tkq_sb, in_=tkq_d[:])
            nc.scalar.dma_start(out=mask_sb, in_=mask_d[:])
            for c in range(2):
                nc.scalar.dma_start(out=wvt_sb[:, c, :], in_=wvt_d[c])
                nc.scalar.dma_start(out=abt_sb[:, c, :], in_=abt_d[c])
            nc.scalar.dma_start(out=id128_sb, in_=id128_d[:])
            nc.scalar.dma_start(out=tvb_sb, in_=tvb_d[:])
            for c in range(2):
                nc.scalar.dma_start(out=wpt_sb[:, c, :], in_=wpt_d[c])

            # persistent double-buffered tiles
            qkT_tiles, qbd_tiles, attnT_tiles = [], [], []
            for i_ in range(2):
                t = stackp.tile([128, 4, 256], BF16, tag=f"qkT{i_}")
                nc.vector.memset(t[:, :, 245:256], 0.0)
                qkT_tiles.append(t)
                qbd_tiles.append([
                    stackp.tile([128, 392], BF16, tag=f"qbd{i_}{j_}",
                                name=f"qbd{i_}{j_}")
                    for j_ in range(4)])
                attnT_tiles.append([
                    stackp.tile([128, 2, 392], BF16, tag=f"at{i_}{j_}",
                                name=f"at{i_}{j_}")
                    for j_ in range(4)])
            hs_tiles = []
            for i_ in range(2):
                hst = stackp.tile([128, 2, 256], BF16, tag=f"hs{i_}")
                nc.vector.memset(hst, 0.0)
                hs_tiles.append(hst)
            v_tiles = []
            for i_ in range(4):
                quad = []
                for j_ in range(4):
                    v0 = stackp.tile([128, 8, 33], BF16, tag=f"v0_{i_}{j_}")
                    v1 = stackp.tile([128, 8, 33], BF16, tag=f"v1_{i_}{j_}")
                    nc.vector.memset(v1, 0.0)
                    nc.vector.memset(v0[:, :, 32:33], 1.0)
                    nc.vector.memset(v1[0:68, :, 32:33], 1.0)
                    quad.append((v0, v1))
                v_tiles.append(quad)

            # identity view with only the useful 98 query columns (fin)
            idq = id128_sb[0:113, :].rearrange(
                "p (two q) -> p two q", two=2)[:, :, 0:49]
            three_sb = consts.tile([128, 1], F32)
            nc.vector.memset(three_sb, 3.0)

            DR = mybir.MatmulPerfMode.DoubleRow

            def xt_load(qd):
                # free dim padded to 208 so the DoubleRow Ko-pair stride
                # (208 fp8 bytes) is 16-byte aligned
                xt_sb = io.tile([128, 8, 208], F8)
                nc.gpsimd.dma_start(
                    out=xt_sb[:, :, 0:196],
                    in_=xt_d[4 * qd:4 * qd + 4].rearrange(
                        "b c q n -> q (b c) n"),
                )
                return xt_sb

            def xs_view(xt_sb, qb):
                v = xt_sb[:, 2 * qb:2 * qb + 2, 0:196].rearrange(
                    "q c (a s c2 t) -> q c a s c2 t", a=7, s=2, c2=7, t=2
                )
                return v[:, :, :, 0, :, 0]

            def a_chunk(qd, xt_sb, qbp):
                """kq + q + v projections for a qb-pair (2 batch items)."""
                qkT_sb = qkT_tiles[qd % 2]
                kq_ps = ps_kq.tile([128, 2, 245], F32)
                for qq in range(2):
                    qb = 2 * qbp + qq
                    nc.tensor.matmul(
                        kq_ps[:, qq, 0:196], lhsT=wkt_sb,
                        rhs=xt_sb[:, 2 * qb:2 * qb + 2, 0:196],
                        start=True, stop=True, perf_mode=DR,
                    )
                    # q-proj without DoubleRow: FWL fp8 weight loads
                    # (2x27ns) beat the DR reload (183ns) at this tiny N
                    xsv = xs_view(xt_sb, qb)
                    for c_ in range(2):
                        nc.tensor.matmul(
                            kq_ps[:, qq, 196:245], lhsT=wqt_sb[:, c_, :],
                            rhs=xsv[:, c_],
                            start=(c_ == 0), stop=(c_ == 1),
                        )
                    nc.vector.tensor_add(
                        qkT_sb[:, qb, 0:245], kq_ps[:, qq, 0:245], tkq_sb)
                    # block-diagonal query tile: bcast q over heads x mask
                    q_ap = qkT_sb[:, qb, 196:245]
                    q_b = bass.AP(tensor=q_ap.tensor, offset=q_ap.offset,
                                  ap=[q_ap.ap[0], [0, 8], q_ap.ap[1]])
                    mask_v = mask_sb.rearrange("p (h q) -> p h q", h=8)
                    qbd_v = qbd_tiles[qd % 2][qb].rearrange(
                        "p (h q) -> p h q", h=8)
                    nc.vector.tensor_mul(qbd_v, q_b, mask_v)

                    v_ps = ps_v.tile([128, 512], F32)
                    nc.tensor.matmul(
                        v_ps[0:128, 0:256],
                        lhsT=xt_sb[:, 2 * qb:2 * qb + 2, 0:128],
                        rhs=wvt_sb,
                        start=True, stop=True, perf_mode=DR,
                    )
                    nc.tensor.matmul(
                        v_ps[0:68, 256:512],
                        lhsT=xt_sb[:, 2 * qb:2 * qb + 2, 128:196],
                        rhs=wvt_sb,
                        start=True, stop=True, perf_mode=DR,
                    )
                    # evac adds WS*t_v: attn@(v+tvb) = o + z*tvb, so the
                    # /z normalization yields y = o/z + tvb exactly and
                    # the gpsimd add leaves the y-chain latency path.
                    # (1/WS scale folded into the y-chain STT.)
                    v0_sb, v1_sb = v_tiles[qd % 4][qb]
                    tvb_v = tvb_sb.rearrange("q (h d) -> q h d", h=8)
                    nc.vector.tensor_add(
                        v0_sb[:, :, 0:32],
                        v_ps[0:128, 0:256].rearrange("q (h d) -> q h d", h=8),
                        tvb_v)
                    nc.vector.tensor_add(
                        v1_sb[0:68, :, 0:32],
                        v_ps[0:68, 256:512].rearrange("q (h d) -> q h d", h=8),
                        tvb_v[0:68])

            def s_chunk(qd, item):
                """transposed scores for one item (both key chunks):
                bias preloaded by identity matmul, qk accumulates on top."""
                qkT_sb = qkT_tiles[qd % 2]
                atts = attnT_tiles[qd % 2]
                qbds = qbd_tiles[qd % 2]
                for c in range(2):
                    s_ps = ps_s.tile([128, 392], F32, tag="s")
                    nc.tensor.matmul(
                        s_ps, lhsT=id128_sb, rhs=abt_sb[:, c, :],
                        start=True, stop=False,
                    )
                    nc.tensor.matmul(
                        s_ps, lhsT=qkT_sb[:, item, 128 * c:128 * c + 128],
                        rhs=qbds[item],
                        start=False, stop=True,
                    )
                    nc.scalar.activation(
                        out=atts[item][:, c, :], in_=s_ps,
                        func=AF.Exp, scale=1.0 / (WS * WS),
                    )

            def av_chunk(qd, av_ps, pr, b2, h0):
                """attn@v for 4 heads of one (pr, b2)."""
                item = 2 * pr + b2
                aT = attnT_tiles[qd % 2][item]
                v0_sb, v1_sb = v_tiles[qd % 4][item]
                for h in range(h0, h0 + 4):
                    for c in range(2):
                        vs = (v0_sb, v1_sb)[c]
                        nc.tensor.matmul(
                            av_ps[64 * b2:64 * b2 + 49, 33 * h:33 * h + 33],
                            lhsT=aT[:, c, 49 * h:49 * h + 49],
                            rhs=vs[:, h, :],
                            start=(c == 0), stop=(c == 1),
                            tile_position=(0, 64 * b2),
                        )

            def y_chain(qd, av_ps, pr):
                """normalization, +tvb, hardswish for one pr half."""
                o_view = av_ps.rearrange("q (h d) -> q h d", h=8)
                zr_sb = work.tile([113, 8], F32, tag=f"tmp{pr}")
                nc.vector.reciprocal(zr_sb, o_view[0:113, :, 32])
                zr_b = bass.AP(tensor=zr_sb.tensor, offset=zr_sb.offset,
                               ap=[zr_sb.ap[0], zr_sb.ap[1], [0, 32]])
                # bf16 y/r: DVE runs 16-bit at 2x, and the ~0.4% rounding
                # is well inside the l2 budget (1.10e-2 vs 2e-2 gate)
                y_sb = work.tile([113, 8, 32], BF16, tag=f"y{pr}")
                nc.vector.scalar_tensor_tensor(
                    out=y_sb, in0=o_view[0:113, :, 0:32], scalar=1.0 / WS,
                    in1=zr_b, op0=ALU.mult, op1=ALU.mult)
                y_flat = y_sb.rearrange("q h d -> q (h d)")
                r_sb = work.tile([113, 256], BF16, tag=f"rr{pr}")
                nc.scalar.activation(
                    r_sb, y_flat, AF.Relu, bias=three_sb[0:113, :], scale=1.0)
                hs_sb = hs_tiles[qd % 2]
                nc.vector.scalar_tensor_tensor(
                    out=hs_sb[0:113, pr, :], in0=r_sb, scalar=6.0,
                    in1=y_flat, op0=ALU.min, op1=ALU.mult,
                )

            def fin_chunk(qd, pj_ps, hs_sb, pr):
                thsT = ps_av.tile([128, 2, 2, 49], BF16, tag="av")
                for cc in range(2):
                    nc.tensor.transpose(
                        thsT[:, cc, :, :],
                        hs_sb[0:113, pr, 128 * cc:128 * cc + 128], idq)
                hsT_sb = work.tile([128, 2, 2, 49], BF16, tag=f"hsT{pr}")
                nc.vector.tensor_copy(hsT_sb, thsT)
                for cc in range(2):
                    for b2 in range(2):
                        nc.tensor.matmul(
                            pj_ps[64 * b2:64 * b2 + 49, pr, 0:512],
                            lhsT=hsT_sb[:, cc, b2, :],
                            rhs=wpt_sb[:, cc, :],
                            start=(cc == 0), stop=(cc == 1),
                            tile_position=(0, 64 * b2),
                        )

            def fin_out(qd, pj_ps):
                out_sb = io.tile([113, 2, 512], BF16)
                nc.scalar.activation(out_sb, pj_ps[0:113, :, :], AF.Copy)
                for b2 in range(2):
                    nc.gpsimd.dma_start(
                        out=outv[b2, qd],
                        in_=out_sb[64 * b2:64 * b2 + 49, :, :])

            def mark(stage, qd):
                STAGE_MARKS.append(
                    (stage, qd,
                     int(nc.get_next_instruction_name().split("-")[1])))

            # skew: A(it)+scores(it) | attn@v(it-1) | fin(it-2).  scores
            # chunks are emitted late in the iteration so the DVE evac +
            # qbd chain they depend on has drained.  Chunks of different
            # stages are interleaved to keep PE duty uniformly high (the
            # HAM MID-window throttles when duty sags).
            nq = nbatch // 4
            x_st, hs_st = {}, {}
            for it in range(nq + 2):
                j, k, m = it, it - 1, it - 2
                do_a, do_s = it < nq, 0 <= j < nq
                do_v, do_f = 0 <= k < nq, 0 <= m < nq
                mark("iter", it)
                if it + 1 < nq:
                    x_st[it + 1] = xt_load(it + 1)
                if do_a and it == 0:
                    x_st[0] = xt_load(0)
                xt_sb = x_st.pop(it) if do_a else None
                if do_v:
                    av0 = ps_av.tile([128, 264], F32, name="av0", tag="av")
                    av1 = ps_av.tile([128, 264], F32, name="av1", tag="av")
                if do_f:
                    pj_ps = ps_pj.tile([113, 2, 512], F32)
                    hs_m = hs_st.pop(m)

                if do_a:
                    a_chunk(it, xt_sb, 0)
                if do_v:
                    av_chunk(k, av0, 0, 0, 0)
                    av_chunk(k, av0, 0, 0, 4)
                if do_a:
                    a_chunk(it, xt_sb, 1)
                if do_v:
                    av_chunk(k, av0, 0, 1, 0)
                    av_chunk(k, av0, 0, 1, 4)
                    y_chain(k, av0, 0)
                if do_f:
                    fin_chunk(m, pj_ps, hs_m, 0)
                if do_s:
                    s_chunk(j, 0)
                if do_v:
                    av_chunk(k, av1, 1, 0, 0)
                if do_s:
                    s_chunk(j, 1)
                if do_v:
                    av_chunk(k, av1, 1, 0, 4)
                    av_chunk(k, av1, 1, 1, 0)
                if do_s:
                    s_chunk(j, 2)
                if do_v:
                    av_chunk(k, av1, 1, 1, 4)
                    y_chain(k, av1, 1)
                    hs_st[k] = hs_tiles[k % 2]
                if do_s:
                    s_chunk(j, 3)
                if do_f:
                    fin_chunk(m, pj_ps, hs_m, 1)
                    fin_out(m, pj_ps)

    STAGE_MARKS.append(("end", -1,
                        int(nc.get_next_instruction_name().split("-")[1])))
    nc.compile()
    return nc


def _build_bias_idxs():
    import itertools
    points = list(itertools.product(range(14), range(14)))
    points_ = list(itertools.product(range(7), range(7)))
    offsets, idxs = {}, []
    for p1 in points_:
        for p2 in points:
            off = (abs(p1[0] * 2 - p2[0]), abs(p1[1] * 2 - p2[1]))
            if off not in offsets:
                offsets[off] = len(offsets)
            idxs.append(offsets[off])
    return np.array(idxs, dtype=np.int32).reshape(NQ, N)


def make_inputs(x, w_kv, kv_g, kv_b, kv_m, kv_v, w_q, q_g, q_b, q_m, q_v,
                w_p, p_g, p_b, p_m, p_v, ab_table, bias_idxs, nbatch=BPC,
                ncores=NCORES):
    """Host-side preprocessing -> list of per-core input dicts."""
    f = np.float32
    x = np.asarray(x, f)
    s_kv = np.asarray(kv_g, f) / np.sqrt(np.asarray(kv_v, f) + EPS)
    wkv = np.asarray(w_kv, f) * s_kv[:, None]
    tkv = np.asarray(kv_b, f) - np.asarray(kv_m, f) * s_kv
    wkv_h = wkv.reshape(H, 48, C)
    tkv_h = tkv.reshape(H, 48)
    w_k = wkv_h[:, :16, :].reshape(128, C)
    t_k = tkv_h[:, :16].reshape(128)
    w_v = wkv_h[:, 16:, :].reshape(256, C)
    t_v = tkv_h[:, 16:].reshape(256)

    s_q = np.asarray(q_g, f) / np.sqrt(np.asarray(q_v, f) + EPS)
    wq = np.asarray(w_q, f) * (s_q * SCALE)[:, None]
    t_q = (np.asarray(q_b, f) - np.asarray(q_m, f) * s_q) * SCALE

    s_p = np.asarray(p_g, f) / np.sqrt(np.asarray(p_v, f) + EPS)
    wp = np.asarray(w_p, f) * s_p[:, None] / 6.0
    t_p = np.asarray(p_b, f) - np.asarray(p_m, f) * s_p

    idxs = _build_bias_idxs()
    WS = 32.0
    ab = np.asarray(ab_table, f)[:, idxs]                       # [8,49,196]
    ab_s = ab * (WS * WS)                                       # [8,49,196]

    # abT[c, k_local, (h,q)] with pad keys (>=196 incl. q-col overlap)
    # biased to -1e6 so exp -> 0
    abt = np.full((2, 128, H, NQ), -1e6, np.float32)
    for c_ in range(2):
        k0 = 128 * c_
        nk = min(196 - k0, 128)
        abt[c_, :nk] = ab_s.transpose(2, 0, 1)[k0:k0 + nk]
    abt = abt.reshape(2, 128, H * NQ)

    mask = np.zeros((128, H * NQ), np.float32)
    for h in range(H):
        mask[16 * h:16 * h + 16, NQ * h:NQ * h + NQ] = 1.0

    tkq = np.concatenate(
        [np.broadcast_to(t_k[:, None], (128, 196)),
         np.broadcast_to(t_q[:, None], (128, 49))], axis=1) * WS

    f8 = ml_dtypes.float8_e4m3fn
    base = dict(
        wkt=np.ascontiguousarray(w_k.T.reshape(2, 128, 128) * WS).astype(f8),
        wqt=np.ascontiguousarray(wq.T.reshape(2, 128, 128) * WS).astype(f8),
        wvt=np.ascontiguousarray(w_v.T.reshape(2, 128, 256) * WS).astype(f8),
        wpt=np.ascontiguousarray(wp.T.reshape(2, 128, 512)).astype(bf16),
        tkq=np.ascontiguousarray(tkq).astype(bf16),
        abt=abt.astype(bf16),
        mask=mask.astype(bf16),
        id128=np.eye(128, dtype=f).astype(bf16),
        tvb=np.ascontiguousarray(np.broadcast_to(t_v * WS, (128, 256))),
    )

    xt = x.transpose(0, 2, 1).astype(f8).reshape(B, 2, 128, N)
    in_maps = []
    for cid in range(ncores):
        m = dict(base)
        m["xt"] = np.ascontiguousarray(xt[cid * nbatch:(cid + 1) * nbatch])
        in_maps.append(m)
    return in_maps, t_p


_NC_CACHE = {}
LAST_RESULT = None


def kernel(**inputs):
    if "nc" not in _NC_CACHE:
        _NC_CACHE["nc"] = build_core(BPC)
    nc = _NC_CACHE["nc"]
    in_maps, t_p = make_inputs(**inputs)
    res = run_bass_kernel_spmd(nc, in_maps, core_ids=list(range(NCORES)))
    global LAST_RESULT
    LAST_RESULT = res
    out = np.concatenate([r["out"] for r in res.results], axis=0)
    return out.astype(np.float32) + t_p
